# revision 6
# baseline (speedup 1.0000x reference)
"""LoKr linear forward on 8 TRN2 NeuronCores — Strassen level-1.

out = x @ (W0 + (alpha/lora_dim) * kron(w1, w2_a @ w2_b)).T + b

Per core: C [2048t, 4096o] = A [2048t, 4096k] @ Bm [4096k, 4096o] + bias,
with A = x token-shard, Bm = W_eff.T folded on host.

Strassen level-1 splits A/Bm/C into 2x2 blocks (t 2048->1024, k 4096->2048,
o 4096->2048) and computes 7 block-products M1..M7 instead of 8:
  M1=(A11+A22)(B11+B22)  M2=(A21+A22)B11  M3=A11(B12-B22)  M4=A22(B21-B11)
  M5=(A11+A12)B22        M6=(A21-A11)(B11+B12)  M7=(A12-A22)(B21+B22)
  C11=M1+M4-M5+M7  C12=M3+M5  C21=M2+M4  C22=M1-M2+M3+M6
PE work drops to 7/8: 3584 matmuls x 216ns = ~775us vs 885us classical.

B-combos are free on the host (7 bf16 [2048,2048] matrices). A-combos are
built on the DVE from streamed xT quarter-slices (bf16 adds). M-products
that are needed later are spilled to DRAM scratch in bf16 and fused back
during the psum eviction of a later phase (phase order M2,M4,M5,M3,M1,M6,M7
completes one C-block in each of the last 4 phases). DMA issue is split
across the sync (B tiles) and scalar (x, spill, out) HWDGE queues.
"""
import sys

sys.path.insert(0, '/opt/trn_rl_repo')

import numpy as np
import ml_dtypes
import concourse.bass as bass
import concourse.mybir as mybir
import concourse.tile as tile
import concourse.bass_utils as bass_utils

ALPHA = 1.0
LORA_DIM = 4
MULTIPLIER = 1.0

N_CORES = 8
B, S, IN, OUT = 4, 4096, 4096, 4096
T_CORE = B * S // N_CORES          # 2048 tokens per core
TH = T_CORE // 2                   # 1024 token half (Strassen block rows)
KH = IN // 2                       # 2048 contraction half
OH = OUT // 2                      # 2048 out-feature half
KT = 128                           # contraction tile (SBUF partitions)
TT = 128                           # token tile (psum partitions)
OT = 512                           # out-feature tile (psum free dim)
NKH = KH // KT                     # 16 k-tiles per product
NTH = TH // TT                     # 8 token tiles per product
NOH = OH // OT                     # 4 o-blocks per product
N_WARMUP = 32

# Phase schedule. Quadrants of xT [4096, 2048]: q=(ka, ta) ->
# xT[ka*KH:(ka+1)*KH, ta*TH:(ta+1)*TH]; A11T=(0,0) A12T=(1,0) A21T=(0,1)
# A22T=(1,1). A-combo = qa + sgn*qb (qb None -> plain slice, no DVE).
# spill: scratch slot for this M; spill_bias: embed bias segment into the
# spill (M4 feeds C21+C11, both in out-cols 0:OH -> carries bias 'lo';
# M3 feeds C12+C22, both in cols OH:2*OH -> carries bias 'hi'; the other
# M products feed C blocks in different column halves, so embedding is
# only sound for these two). fuse: C-block completed in this phase:
#   tr/oc: token-half/out-half; lds: [(slot, sign), ...]
#   C = psum + sum(sign * spill[slot]) (+ explicit bias where flagged)
# slots: 0=M2 1=M4(+bias_lo) 2=M5 3=M3(+bias_hi) 4=M1
# First phase is a direct one: cold-start MMs gate only on plain x DMAs.
PHASES = [
    dict(mi=3, qa=(0, 0), qb=None, sgn=+1, spill=3, spill_bias=1,
         fuse=None),
    dict(mi=2, qa=(0, 1), qb=(1, 1), sgn=+1, spill=0, spill_bias=None,
         fuse=None),
    dict(mi=5, qa=(0, 0), qb=(1, 0), sgn=+1, spill=2, spill_bias=None,
         fuse=dict(tr=0, oc=1, lds=[(3, +1)], bias=False)),    # C12=M5+S3
    dict(mi=4, qa=(1, 1), qb=None, sgn=+1, spill=1, spill_bias=0,
         fuse=dict(tr=1, oc=0, lds=[(0, +1)], bias='spill')),  # C21=M4+b+S0
    dict(mi=1, qa=(0, 0), qb=(1, 1), sgn=+1, spill=4, spill_bias=None,
         fuse=None),
    dict(mi=6, qa=(0, 1), qb=(0, 0), sgn=-1, spill=None, spill_bias=None,
         fuse=dict(tr=1, oc=1, lds=[(4, +1), (3, +1), (0, -1)],
                   bias=False)),                               # C22
    dict(mi=7, qa=(1, 0), qb=(1, 1), sgn=-1, spill=None, spill_bias=None,
         fuse=dict(tr=0, oc=0, lds=[(4, +1), (1, +1), (2, -1)],
                   bias=False)),                               # C11 (b in S1)
]


def _split_multi_waits(nc):
    """This walrus build encodes at most ONE semaphore wait per ISA
    instruction; hoist extra waits onto single-wait NOPs inserted before."""
    ctr = 0
    for f in nc.m.functions:
        for blk in f.blocks:
            out = []
            changed = False
            for i in blk.instructions:
                si = i.sync_info
                if si is not None and si.on_wait and len(si.on_wait) > 1:
                    waits = list(si.on_wait)
                    for w in waits[:-1]:
                        ctr += 1
                        out.append(mybir.InstNoOp(
                            name=f"I-wsplit-{ctr}",
                            engine=i.engine, ins=[], outs=[],
                            sync_info=mybir.SyncInfo(on_wait=[w], on_update=[]),
                        ))
                    i.sync_info = mybir.SyncInfo(
                        on_wait=[waits[-1]], on_update=list(si.on_update))
                    changed = True
                out.append(i)
            if changed:
                blk.instructions = out


def build_nc():
    nc = bass.Bass(trn_type="TRN2")
    bf16 = mybir.dt.bfloat16
    f32 = mybir.dt.float32
    xT = nc.dram_tensor("xT", [IN, T_CORE], bf16, kind="ExternalInput")
    BC = nc.dram_tensor("BC", [7 * KH, OH], bf16, kind="ExternalInput")
    bias_d = nc.dram_tensor("bias", [128, OUT], bf16, kind="ExternalInput")
    out = nc.dram_tensor("out", [T_CORE, OUT], bf16, kind="ExternalOutput")

    with tile.TileContext(nc) as tc:
        with (
            tc.tile_pool(name="const", bufs=1) as constp,
            tc.tile_pool(name="warm", bufs=1) as warmp,
            tc.tile_pool(name="ac", bufs=2 * NKH) as acp,
            tc.tile_pool(name="xs", bufs=14) as xsp,
            tc.tile_pool(name="wb", bufs=2 * NKH) as wbp,
            tc.tile_pool(name="sp", bufs=2) as spp,
            tc.tile_pool(name="ld", bufs=5) as ldp,
            tc.tile_pool(name="ut", bufs=6) as utp,
            tc.tile_pool(name="ot", bufs=4) as otp,
            tc.tile_pool(name="ps", bufs=8, space="PSUM") as pp,
            tc.tile_pool(name="scr", bufs=5 * NOH, space="DRAM") as scrp,
        ):
            # DRAM scratch for spilled M products, bf16. One tile per
            # (slot, o-block) so spill->reload dependencies stay exact
            # (a reload only waits on the one spill DMA that produced it).
            S = {(slot, o): scrp.tile([128, NTH * OT], bf16,
                                      name=f"S_{slot}_{o}")
                 for slot in range(5) for o in range(NOH)}

            # PE warm-up from t~=0 (no DMA dependency): opens the HAM
            # clock gate while DMA rings come up and first tiles stream.
            wz = warmp.tile([KT, OT], bf16)
            nc.vector.memset(wz[:], 0)
            wps = pp.tile([TT, OT], f32, tag="ps")
            for _ in range(N_WARMUP):
                nc.tensor.matmul(wps[:], wz[:, :TT], wz[:],
                                 start=True, stop=True)

            bias = constp.tile([128, OUT], bf16)

            def build_combo(ph):
                """A-combo tiles for a phase: 16 x [128, TH] bf16.
                Issues x DMAs (scalar queue); returns (tiles, dve_thunks) --
                dve_thunks are deferred so combo DVE ops can be interleaved
                between eviction bursts (keeps the DVE FIFO head unblocked)."""
                tiles, thunks = [], []
                ka, ta = ph['qa']
                for kt in range(NKH):
                    act = acp.tile([KT, TH], bf16, tag="ac")
                    ra = ka * KH + kt * KT
                    if ph['qb'] is None:
                        nc.scalar.dma_start(
                            act[:], xT[ra:ra + KT, ta * TH:(ta + 1) * TH])
                    else:
                        kb, tb = ph['qb']
                        rb = kb * KH + kt * KT
                        xa = xsp.tile([KT, TH], bf16, tag="xs")
                        nc.scalar.dma_start(
                            xa[:], xT[ra:ra + KT, ta * TH:(ta + 1) * TH])
                        xb = xsp.tile([KT, TH], bf16, tag="xs")
                        nc.scalar.dma_start(
                            xb[:], xT[rb:rb + KT, tb * TH:(tb + 1) * TH])
                        op = (nc.vector.tensor_add if ph['sgn'] > 0
                              else nc.vector.tensor_sub)
                        thunks.append(
                            (lambda op=op, a=act, x1=xa, x2=xb:
                             op(a[:], x1[:], x2[:])))
                    tiles.append(act)
                return tiles, thunks

            def load_lds(ph, o):
                """Spill-read tiles for a fuse phase's o-block: one
                [128, NTH*OT] bf16 tile per needed slot, 4 quarter DMAs."""
                lds = {}
                if ph['fuse'] is None:
                    return lds
                for slot, _sgn in ph['fuse']['lds']:
                    ld = ldp.tile([128, NTH * OT], bf16, tag="ld")
                    for q in range(4):
                        c0 = q * (NTH * OT // 4)
                        c1 = c0 + NTH * OT // 4
                        nc.scalar.dma_start(
                            ld[:, c0:c1], S[slot, o][:, c0:c1])
                    lds[slot] = ld
                return lds

            def evict(ph, o, t, ps, sp, lds):
                f = ph['fuse']
                spill_done = False
                if f is not None:
                    ocol = f['oc'] * OH + o * OT
                    trow = f['tr'] * TH + t * TT
                    ldl = f['lds']
                    tsl = slice(t * OT, (t + 1) * OT)
                    ot_t = otp.tile([TT, OT], bf16, tag="ot")
                    if f['bias'] == 'spill':
                        # C21: write own biased spill first, then reuse it.
                        sb = ph['spill_bias'] * OH + o * OT
                        nc.vector.tensor_add(
                            sp[:, tsl], ps[:], bias[:, sb:sb + OT])
                        spill_done = True
                        (slot, sgn), = ldl
                        assert sgn > 0
                        nc.vector.tensor_add(
                            ot_t[:], sp[:, tsl], lds[slot][:, tsl])
                    elif len(ldl) == 1:
                        # C12: psum + (M3+bias_hi) spill, single op.
                        (slot, sgn), = ldl
                        assert sgn > 0
                        nc.vector.tensor_add(
                            ot_t[:], ps[:], lds[slot][:, tsl])
                    else:
                        # C22/C11: psum + ld0 + (ld1 - ld2); bias embedded.
                        (s0, g0), (s1, g1), (s2, g2) = ldl
                        assert g0 > 0 and g1 > 0 and g2 < 0
                        u1 = utp.tile([TT, OT], f32, tag="ut")
                        nc.vector.tensor_sub(
                            u1[:], lds[s1][:, tsl], lds[s2][:, tsl])
                        u2 = utp.tile([TT, OT], f32, tag="ut")
                        nc.vector.tensor_add(
                            u2[:], ps[:], lds[s0][:, tsl])
                        nc.vector.tensor_add(ot_t[:], u2[:], u1[:])
                    nc.scalar.dma_start(
                        out[trow:trow + TT, ocol:ocol + OT], ot_t[:])
                if ph['spill'] is not None and not spill_done:
                    tsl = slice(t * OT, (t + 1) * OT)
                    if ph['spill_bias'] is not None:
                        sb = ph['spill_bias'] * OH + o * OT
                        nc.vector.tensor_add(
                            sp[:, tsl], ps[:], bias[:, sb:sb + OT])
                    else:
                        nc.vector.tensor_copy(sp[:, tsl], ps[:])

            def spill_out(ph, o, sp):
                if ph['spill'] is None:
                    return
                for q in range(4):
                    c0 = q * (NTH * OT // 4)
                    c1 = c0 + NTH * OT // 4
                    nc.scalar.dma_start(
                        S[ph['spill'], o][:, c0:c1], sp[:, c0:c1])

            # bias (bf16) in 512-col slices, issued before anything else:
            # every DMA issued in the first ~10us pays the ~26us DMA-ring
            # bring-up, and phase 0's first biased spill eviction needs
            # bias cols 2048:2560 by ~45us -- so that slice goes absolutely
            # first, followed by the rest of the hi half, then the lo half.
            for c0 in (2048, 2560, 3072, 3584, 0, 512, 1024, 1536):
                nc.sync.dma_start(bias[:, c0:c0 + OT],
                                  bias_d[:, c0:c0 + OT])
            combo, combo_thunks = build_combo(PHASES[0])
            for th in combo_thunks:
                th()
            pending_lds = load_lds(PHASES[0], 0)
            for pi, ph in enumerate(PHASES):
                mi0 = (ph['mi'] - 1) * KH
                lds = pending_lds
                nph = PHASES[pi + 1] if pi + 1 < len(PHASES) else None
                # prefetch next phase's A-combo inputs (scalar queue); its
                # DVE ops run interleaved with o-block evictions. During
                # the cold phase 0 the DMA system is saturated with phase
                # 0's own tiles, so the prefetch is deferred to o-block 1
                # and the combo ops to o-blocks 2-3.
                ncombo, nthunks = None, []
                if nph is not None and pi > 0:
                    ncombo, nthunks = build_combo(nph)
                for o in range(NOH):
                    if pi == 0 and o == 1 and nph is not None:
                        ncombo, nthunks = build_combo(nph)
                    wts = []
                    for kt in range(NKH):
                        wt = wbp.tile([KT, OT], bf16, tag="wb")
                        nc.sync.dma_start(
                            wt[:], BC[mi0 + kt * KT:mi0 + (kt + 1) * KT,
                                      o * OT:(o + 1) * OT])
                        wts.append(wt)
                    if o + 1 < NOH:
                        nlds = load_lds(ph, o + 1)
                    else:
                        # prefetch the NEXT phase's first spill-reads one
                        # o-block early so its first evictions are data-ready
                        nlds = None
                        if nph is not None:
                            pending_lds = load_lds(nph, 0)
                    sp = None
                    if ph['spill'] is not None:
                        sp = spp.tile([128, NTH * OT], bf16, tag="sp")
                    # Next-phase combo DVE ops are interleaved one per
                    # eviction position: phases >0 run them across o-blocks
                    # 0-1 (x-DMAs issued at phase start are data-ready and
                    # the pacing keeps the xs pool flowing so the scalar
                    # DMA queue never head-blocks); phase 0 runs them
                    # across o-blocks 2-3 (x prefetch deferred to o1).
                    def thunk_idx(t, o=o):
                        if pi == 0:
                            return (o - 2) * NTH + t if o >= 2 else -1
                        return o * NTH + t if o < 2 else -1
                    if pi == 0 and o == 0:
                        # cold start: k-outer/t-inner across all 8 psum
                        # banks tracks the DMA wavefront of the very first
                        # combo+B tiles instead of stalling on one chain.
                        pss = [pp.tile([TT, OT], f32, tag="ps",
                                       name=f"pss_{i}") for i in range(NTH)]
                        for kt in range(NKH):
                            for t in range(NTH):
                                nc.tensor.matmul(
                                    pss[t][:],
                                    combo[kt][:, t * TT:(t + 1) * TT],
                                    wts[kt][:],
                                    start=(kt == 0), stop=(kt == NKH - 1))
                                if kt == NKH - 1:
                                    evict(ph, o, t, pss[t], sp, lds)
                    else:
                        for t in range(NTH):
                            ps = pp.tile([TT, OT], f32, tag="ps")
                            for kt in range(NKH):
                                nc.tensor.matmul(
                                    ps[:],
                                    combo[kt][:, t * TT:(t + 1) * TT],
                                    wts[kt][:],
                                    start=(kt == 0), stop=(kt == NKH - 1))
                            evict(ph, o, t, ps, sp, lds)
                            ti = thunk_idx(t)
                            if 0 <= ti < len(nthunks):
                                nthunks[ti]()
                    spill_out(ph, o, sp)
                    lds = nlds
                combo, combo_thunks = ncombo, nthunks
    _split_multi_waits(nc)
    return nc


_NC_CACHE = []


def _get_nc():
    if not _NC_CACHE:
        _NC_CACHE.append(build_nc())
    return _NC_CACHE[0]


def make_in_maps(x, W0, b, lokr_w1, lokr_w2_a, lokr_w2_b):
    scale = (ALPHA / LORA_DIM) * MULTIPLIER
    w2 = lokr_w2_a.astype(np.float32) @ lokr_w2_b.astype(np.float32)
    w_eff = W0.astype(np.float32) + scale * np.kron(
        lokr_w1.astype(np.float32), w2)
    Bm = np.ascontiguousarray(w_eff.T)          # [k, o] f32
    B11 = Bm[:KH, :OH]
    B12 = Bm[:KH, OH:]
    B21 = Bm[KH:, :OH]
    B22 = Bm[KH:, OH:]
    combos = [B11 + B22, B11, B12 - B22, B21 - B11, B22,
              B11 + B12, B21 + B22]
    BC = np.ascontiguousarray(
        np.concatenate(combos, axis=0)).astype(ml_dtypes.bfloat16)
    bias_rep = np.ascontiguousarray(
        np.broadcast_to(b.astype(np.float32)[None, :],
                        (128, OUT))).astype(ml_dtypes.bfloat16)
    xs = x.astype(np.float32).reshape(B * S, IN)
    in_maps = []
    for c in range(N_CORES):
        shard = xs[c * T_CORE:(c + 1) * T_CORE]
        xT_bf = np.ascontiguousarray(shard.T).astype(ml_dtypes.bfloat16)
        in_maps.append({"xT": xT_bf, "BC": BC, "bias": bias_rep})
    return in_maps


def run_spmd(in_maps, trace=False, **kw):
    nc = _get_nc()
    return bass_utils.run_bass_kernel_spmd(
        nc, in_maps, core_ids=list(range(N_CORES)), trace=trace, **kw)


def kernel(x, W0, b, lokr_w1, lokr_w2_a, lokr_w2_b):
    in_maps = make_in_maps(x, W0, b, lokr_w1, lokr_w2_a, lokr_w2_b)
    res = run_spmd(in_maps, trace=False)
    out = np.concatenate(
        [np.asarray(res.results[c]["out"]).astype(np.float32)
         for c in range(N_CORES)], axis=0)
    return out.reshape(B, S, OUT).astype(np.float32)
